# revision 1
# baseline (speedup 1.0000x reference)
"""Trainium2 Bass kernel: 16-head self-attention (B=2, N=2048, C=1024) on 8 cores.

Sharding: core c -> (batch b = c//4, head-group g = c%4 owning heads 4g..4g+3).
Each core computes QKV projection for its heads, full softmax attention, and a
partial out-projection (its heads' input-channel slice of W_out); the host sums
the 4 partials per batch (tensor-parallel all-reduce done on host at gather).

On-chip layout choices:
  - q,k produced TRANSPOSED [D, N] (channels on partitions) so scores are
    computed as S^T[key, q] = kT.T @ qT with keys on PSUM partitions.
  - v produced NATURAL [N, D] with a ones-column appended, so O^T = (V|1).T @ P^T
    yields the softmax denominators as row 64 of the accumulator for free.
  - exp on ScalarE reads S^T from PSUM in [128,1024] chunks:
    P = exp(scale*S + shift); the constant shift cancels in normalization.
  - all matmuls run in float32r (TF32-like, full PE rate at N>=256).
"""
import os

import numpy as np

B, N, C, H, D = 2, 2048, 1024, 16, 64
HPC = 4            # heads per core
SCALE = float(D) ** -0.5
SHIFT = -12.0      # exp arg shift; scores observed in [-9.1, 9.1] scaled
P = 128
NT = N // 512      # 4 token chunks of 512
KT = N // 128      # 16 key tiles

_cache = {}


def _build_nc():
    import concourse.bass as bass  # noqa: F401
    import concourse.mybir as mybir
    from concourse import bacc
    from concourse.tile import TileContext

    f32 = mybir.dt.float32
    f32r = mybir.dt.float32r
    bf16 = mybir.dt.bfloat16
    Exp = mybir.ActivationFunctionType.Exp
    mult = mybir.AluOpType.mult

    nc = bacc.Bacc("TRN2", target_bir_lowering=False, debug=False, num_devices=8)

    xT = nc.dram_tensor("xT", [P, 8, N], f32r, kind="ExternalInput")
    wqk = nc.dram_tensor("wqk", [P, 8, 512], f32r, kind="ExternalInput")
    wv = nc.dram_tensor("wv", [P, 8, 256], f32r, kind="ExternalInput")
    wo = nc.dram_tensor("wo", [P, 2, 1024], f32r, kind="ExternalInput")
    bqk = nc.dram_tensor("bqk", [P, 4], f32, kind="ExternalInput")
    out_y = nc.dram_tensor("out_y", [N, C], f32, kind="ExternalOutput")

    with TileContext(nc) as tc:
        with tc.tile_pool(name="sb", bufs=1) as sb, \
             tc.tile_pool(name="ps", bufs=1, space="PSUM") as ps:
            # ---- weights + persistent tiles ----
            wqk_sb = sb.tile([P, 8, 512], f32r)
            nc.sync.dma_start(wqk_sb, wqk[:])
            wv_sb = sb.tile([P, 8, 256], f32r)
            nc.sync.dma_start(wv_sb, wv[:])
            wo_sb = sb.tile([P, 2, 1024], f32r)
            nc.sync.dma_start(wo_sb, wo[:])
            bqk_sb = sb.tile([P, 4], f32)
            nc.sync.dma_start(bqk_sb, bqk[:])

            xT_sb = sb.tile([P, 8, N], f32r)
            qkT_sb = sb.tile([P, 4, N], f32r)       # [q01|q23|k01|k23] x tokens
            v_sb = sb.tile([P, KT, HPC, 65], bf16)  # tokens x (head, D|ones)
            oT_sb = sb.tile([P, 2, N], f32r)        # head channels x q

            ones_f = sb.tile([P, 1], f32)
            nc.vector.memset(ones_f, 1.0)
            with nc.allow_low_precision(reason="exact 1.0 to f32r"):
                nc.vector.tensor_copy(v_sb[:, :, :, 64:65],
                                      ones_f[:, 0:1, None].to_broadcast((P, KT, HPC, 1)))
            ones_r = sb.tile([1, 64], f32r)
            with nc.allow_low_precision(reason="exact 1.0 to f32r"):
                nc.vector.tensor_copy(ones_r, ones_f[0:1, :].to_broadcast((1, 64)))
            shift_sb = sb.tile([P, 1], f32)
            nc.vector.memset(shift_sb, SHIFT)

            # ---- phase 1 + 2 interleaved: QKV chains just-in-time with attention ----
            def qk_chain(ms, nt):
                tok = slice(nt * 512, (nt + 1) * 512)
                qk_ps = ps.tile([P, 1024], f32, tag="s", bufs=2, name="qk_ps")[:, 0:512]
                for ks in range(8):
                    nc.tensor.matmul(
                        qk_ps,
                        lhsT=wqk_sb[:, ks, ms * 128:(ms + 1) * 128],
                        rhs=xT_sb[:, ks, tok],
                        start=(ks == 0), stop=(ks == 7),
                    )
                with nc.allow_low_precision(reason="qkT f32r for PE"):
                    nc.vector.tensor_scalar_add(
                        qkT_sb[:, ms, tok], qk_ps, bqk_sb[:, ms:ms + 1])

            def v_chain(kt):
                v_ps = ps.tile([P, 1024], f32, tag="s", bufs=2, name="v_ps")[:, 0:256]
                for ks in range(8):
                    nc.tensor.matmul(
                        v_ps,
                        lhsT=xT_sb[:, ks, kt * 128:(kt + 1) * 128],
                        rhs=wv_sb[:, ks, :],
                        start=(ks == 0), stop=(ks == 7),
                    )
                with nc.allow_low_precision(reason="v f32r for PE"):
                    nc.vector.tensor_copy(
                        v_sb[:, kt, :, 0:64],
                        v_ps.rearrange("p (h d) -> p h d", h=HPC))

            for half in range(8):        # finer DMA chunks so chain 0 starts early
                nc.sync.dma_start(xT_sb[:, :, half * 256:(half + 1) * 256],
                                  xT[:, :, half * 256:(half + 1) * 256])
            qk_chain(0, 0)               # minimal prefix before head-0 qh0
            qk_chain(0, 1)
            qk_chain(2, 0)               # k01 nt0
            v_chain(0)
            # just-in-time emission of remaining projection chains inside head 0:
            # qh=1 work first appears within kt=0, so q01(2,3) must come at kt 0.
            jit_chains = {
                0:  [(0, 2), (0, 3), ("v", 1)],
                1:  [("v", 2), ("v", 3)],
                4:  [(2, 1)] + [("v", kt) for kt in range(4, 8)] + [(1, 0)],
                8:  [(2, 2)] + [("v", kt) for kt in range(8, 12)] + [(1, 1), (3, 0)],
                12: [(2, 3)] + [("v", kt) for kt in range(12, 16)] + [(1, 2), (3, 1)],
            }
            tail_chains = [(1, 3), (3, 2), (3, 3)]

            # ---- phase 2: attention per head ----
            for h in range(HPC):
                qsub, hp = h // 2, 64 * (h % 2)
                ksub = 2 + h // 2
                o_ps = []
                for qh in range(2):
                    o_t = ps.tile([P, 1024], f32, tag="acc", bufs=2, name=f"o_ps{qh}")
                    o_ps.append(o_t)
                for kt in range(KT):
                    if h == 0 and kt != 0:
                        for item in jit_chains.get(kt, []):
                            if item[0] == "v":
                                v_chain(item[1])
                            else:
                                qk_chain(item[0], item[1])
                    key = slice(kt * 128, (kt + 1) * 128)
                    for qh in range(2):
                        if h == 0 and kt == 0 and qh == 1:
                            for item in jit_chains[0]:
                                if item[0] == "v":
                                    v_chain(item[1])
                                else:
                                    qk_chain(item[0], item[1])
                        s_ps = ps.tile([P, 1024], f32, tag="s", bufs=2, name="s_ps")
                        for j in range(2):
                            qs = slice(qh * 1024 + j * 512, qh * 1024 + (j + 1) * 512)
                            nc.tensor.matmul(
                                s_ps[:, j * 512:(j + 1) * 512],
                                lhsT=qkT_sb[hp:hp + 64, ksub, key],
                                rhs=qkT_sb[hp:hp + 64, qsub, qs],
                                start=True, stop=True,
                            )
                        pT = sb.tile([P, 1024], bf16, tag="pT", bufs=3, name="pT")
                        nc.scalar.activation(pT, s_ps, Exp, bias=shift_sb, scale=SCALE)
                        for j in range(2):
                            nc.tensor.matmul(
                                o_ps[qh][0:65, j * 512:(j + 1) * 512],
                                lhsT=v_sb[:, kt, h, :],
                                rhs=pT[:, j * 512:(j + 1) * 512],
                                start=(kt == 0), stop=(kt == KT - 1),
                            )
                if h == 0:
                    for item in tail_chains:
                        qk_chain(item[0], item[1])
                # normalization: r = 1/sums (row 64), broadcast via ones-matmul
                r_sb = sb.tile([1, N], f32r, tag="r", bufs=2, name="r_sb")
                with nc.allow_low_precision(reason="softmax denom recip"):
                    for qh in range(2):
                        nc.vector.reciprocal(
                            r_sb[0:1, qh * 1024:(qh + 1) * 1024], o_ps[qh][64:65, :])
                for qh in range(2):
                    rb_ps = ps.tile([P, 1024], f32, tag="s", bufs=2, name="rb_ps")
                    for j in range(2):
                        nc.tensor.matmul(
                            rb_ps[0:64, j * 512:(j + 1) * 512], lhsT=ones_r,
                            rhs=r_sb[0:1, qh * 1024 + j * 512:qh * 1024 + (j + 1) * 512],
                            start=True, stop=True,
                        )
                    rbc_sb = sb.tile([64, 1024], f32, tag="rbc", bufs=2, name="rbc_sb")
                    nc.vector.tensor_copy(rbc_sb, rb_ps[0:64, :])
                    with nc.allow_low_precision(reason="oT f32r for PE"):
                        nc.vector.tensor_tensor(
                            out=oT_sb[hp:hp + 64, qsub, qh * 1024:(qh + 1) * 1024],
                            in0=o_ps[qh][0:64, :],
                            in1=rbc_sb,
                            op=mult,
                        )

            # ---- phase 3: out-projection (partial over this core's channels) ----
            for qt in range(KT):
                for n2 in range(2):
                    y_ps = ps.tile([P, 1024], f32, tag="s", bufs=2, name="y_ps")[:, 0:512]
                    for ks2 in range(2):
                        nc.tensor.matmul(
                            y_ps,
                            lhsT=oT_sb[:, ks2, qt * 128:(qt + 1) * 128],
                            rhs=wo_sb[:, ks2, n2 * 512:(n2 + 1) * 512],
                            start=(ks2 == 0), stop=(ks2 == 1),
                        )
                    y_sb = sb.tile([P, 512], f32, tag="y", bufs=3, name="y_sb")
                    nc.vector.tensor_copy(y_sb, y_ps)
                    nc.sync.dma_start(
                        out_y[qt * 128:(qt + 1) * 128, n2 * 512:(n2 + 1) * 512], y_sb)

    nc.compile()
    return nc


def _get_nc():
    if "nc" not in _cache:
        _cache["nc"] = _build_nc()
    return _cache["nc"]


def kernel(x, W_in, b_in, W_out, b_out):
    from concourse.bass_utils import run_bass_kernel_spmd

    x = np.asarray(x, dtype=np.float32)
    W_in = np.asarray(W_in, dtype=np.float32)
    b_in = np.asarray(b_in, dtype=np.float32)
    W_out = np.asarray(W_out, dtype=np.float32)
    b_out = np.asarray(b_out, dtype=np.float32)

    in_maps = []
    for c in range(8):
        b, g = c // 4, c % 4
        rs = slice(256 * g, 256 * g + 256)

        xTc = np.ascontiguousarray(
            x[b].T.reshape(8, 128, N).transpose(1, 0, 2))          # [128,8,N]
        Wqk = np.concatenate([W_in[0:C][rs], W_in[C:2 * C][rs]])   # [512,1024]
        wqkc = np.ascontiguousarray(
            Wqk.T.reshape(8, 128, 512).transpose(1, 0, 2))         # [128,8,512]
        Wv = W_in[2 * C:3 * C][rs]                                 # [256,1024]
        wvc = np.ascontiguousarray(
            Wv.T.reshape(8, 128, 256).transpose(1, 0, 2))          # [128,8,256]
        WoT = np.ascontiguousarray(W_out[:, rs].T)                 # [256,1024]
        woc = np.ascontiguousarray(WoT.reshape(2, 128, 1024).transpose(1, 0, 2))
        bqkc = np.ascontiguousarray(
            np.concatenate([b_in[0:C][rs], b_in[C:2 * C][rs]]).reshape(4, 128).T)

        in_maps.append({"xT": xTc, "wqk": wqkc, "wv": wvc, "wo": woc, "bqk": bqkc})

    nc = _get_nc()
    trace = os.environ.get("KERNEL_TRACE", "0") == "1"
    bkr = run_bass_kernel_spmd(nc, in_maps, core_ids=list(range(8)), trace=trace)
    _cache["last_bkr"] = bkr
    res = bkr.results

    y = np.zeros((B, N, C), dtype=np.float32)
    for c in range(8):
        y[c // 4] += res[c]["out_y"]
    # v-bias folds through softmax (rows sum to 1) and out-proj exactly
    y += (b_in[2 * C:3 * C] @ W_out.T + b_out)[None, None, :]
    return y



# revision 8
# speedup vs baseline: 1.6968x; 1.6968x over previous
"""Trainium2 Bass kernel: 16-head self-attention (B=2, N=2048, C=1024) on 8 cores.

Sharding: core c -> (batch b = c//4, head-group g = c%4 owning heads 4g..4g+3).
Each core computes QKV projection for its heads, full softmax attention, and a
partial out-projection (its heads' input-channel slice of W_out); the host sums
the 4 partials per batch (tensor-parallel all-reduce done on host at gather).

v2 design (vs the first working version, which measured 427us):
  - Head PAIRS processed together with PE row-tiling: the two heads of a pair
    keep their 64 d-channels on partitions 0-63 / 64-127, so their K=64 score
    matmuls run CONCURRENTLY on the two row halves of the PE array.
  - Query-block-major loop (4 blocks of 512 queries): the out-projection of
    block b runs inside block b+1's attention window instead of a serial tail.
  - Softmax denominator reciprocals: [1,512] rows are spread to [128,8] via a
    DRAM round-trip so the DVE reciprocal microcode runs across 128 lanes
    (~0.1us) instead of serializing in one lane (6.5us per call in v1).
  - o accumulators are evacuated PSUM->SBUF immediately (one DVE copy) so the
    single accumulator bank pair can be reused by the next head pair at once;
    normalization (broadcast-matmul + multiply) happens later off SBUF.
  - Emission is software-pipelined: PV runs SKEW slots behind scores/exp, and
    projection / out-projection / normalization quanta are placed in fixed
    slots so the Scalar engine's exp stream (the 128us floor) never starves
    and the PE never idles long enough to re-throttle (HAM).
  - exp on ScalarE reads S^T from PSUM as [128,1024] tiles (head A's 512
    queries | head B's 512): P = exp(scale*S + shift); the constant shift
    cancels in normalization.  All matmuls in float32r (full rate at N>=256);
    probabilities and V in bf16.
"""
import os

import numpy as np

B, N, C, H, D = 2, 2048, 1024, 16, 64
HPC = 4            # heads per core
SCALE = float(D) ** -0.5
SHIFT = -12.0      # exp arg shift; scores observed in [-9.1, 9.1] scaled
P = 128
KT = N // 128      # 16 key tiles
QB = N // 512      # 4 query blocks
SKEW = 4           # PV emission lag (slots) behind scores/exp

_cache = {}


def _build_nc():
    import concourse.bass as bass  # noqa: F401
    import concourse.mybir as mybir
    from concourse import bacc
    from concourse.tile import TileContext

    f32 = mybir.dt.float32
    f32r = mybir.dt.float32r
    bf16 = mybir.dt.bfloat16
    Exp = mybir.ActivationFunctionType.Exp
    mult = mybir.AluOpType.mult

    nc = bacc.Bacc("TRN2", target_bir_lowering=False, debug=False, num_devices=8)

    xT = nc.dram_tensor("xT", [P, 8, N], f32r, kind="ExternalInput")
    wqk = nc.dram_tensor("wqk", [P, 8, 512], f32r, kind="ExternalInput")
    wv = nc.dram_tensor("wv", [P, 8, 256], f32r, kind="ExternalInput")
    wo = nc.dram_tensor("wo", [P, 2, 1024], f32r, kind="ExternalInput")
    bqk = nc.dram_tensor("bqk", [P, 4], f32, kind="ExternalInput")
    out_y = nc.dram_tensor("out_y", [N, C], f32, kind="ExternalOutput")

    with TileContext(nc) as tc:
        with tc.tile_pool(name="sb", bufs=1) as sb, \
             tc.tile_pool(name="ps", bufs=1, space="PSUM") as ps:
            # ---- persistent SBUF ----
            wqk_sb = sb.tile([P, 8, 512], f32r)
            wv_sb = sb.tile([P, 8, 256], f32r)
            wo_sb = sb.tile([P, 2, 1024], f32r)
            bqk_sb = sb.tile([P, 4], f32)
            xT_sb = sb.tile([P, 8, N], f32r)
            qkT_sb = sb.tile([P, 4, N], f32r)       # [q01|q23|k01|k23] x tokens
            v_sb = sb.tile([P, KT, HPC, 65], bf16)  # tokens x (head, D|ones)
            oT_sb = sb.tile([P, 2, N], f32r)        # head channels x tokens
            dS = sb.tile([P, 8], f32)               # denoms spread over lanes
            rS = sb.tile([P, 8], f32r)              # reciprocals, spread
            r2 = sb.tile([1, 1024], f32r)           # reciprocals, heads on free

            # DMA order tuned so just-in-time chains see their inputs arrive:
            # ramp needs wqk + tokens 0..511; v chains need wv early.
            nc.sync.dma_start(wqk_sb, wqk[:])
            nc.sync.dma_start(wv_sb, wv[:])
            for c in range(2):
                nc.sync.dma_start(xT_sb[:, :, c * 256:(c + 1) * 256],
                                  xT[:, :, c * 256:(c + 1) * 256])
            nc.sync.dma_start(bqk_sb, bqk[:])
            for c in range(2, 8):
                nc.sync.dma_start(xT_sb[:, :, c * 256:(c + 1) * 256],
                                  xT[:, :, c * 256:(c + 1) * 256])
            nc.sync.dma_start(wo_sb, wo[:])

            ones_f = sb.tile([P, 1], f32)
            nc.vector.memset(ones_f, 1.0)
            with nc.allow_low_precision(reason="exact 1.0 to bf16"):
                nc.vector.tensor_copy(v_sb[:, :, :, 64:65],
                                      ones_f[:, 0:1, None].to_broadcast((P, KT, HPC, 1)))
            ones_r = sb.tile([1, 64], f32r)
            with nc.allow_low_precision(reason="exact 1.0 to f32r"):
                nc.vector.tensor_copy(ones_r, ones_f[0:1, :].to_broadcast((1, 64)))
            shift_sb = sb.tile([P, 1], f32)
            nc.vector.memset(shift_sb, SHIFT)

            # ---- PE slack fillers ----
            qk_open = {}

            def qk_half(ms, nt, half):
                # K=1024 contraction in 8 steps; emitted as two 4-step halves
                # so a chain never monopolizes the PE for >1us.
                tok = slice(nt * 512, (nt + 1) * 512)
                if half == 0:
                    qk_open[(ms, nt)] = ps.tile([P, 512], f32, tag="c", bufs=1,
                                                name="qk_ps")
                qk_ps = qk_open[(ms, nt)]
                for ks in range(4 * half, 4 * half + 4):
                    nc.tensor.matmul(
                        qk_ps,
                        lhsT=wqk_sb[:, ks, ms * 128:(ms + 1) * 128],
                        rhs=xT_sb[:, ks, tok],
                        start=(ks == 0), stop=(ks == 7),
                    )
                if half == 1:
                    with nc.allow_low_precision(reason="qkT f32r for PE"):
                        nc.vector.tensor_scalar_add(
                            qkT_sb[:, ms, tok], qk_ps, bqk_sb[:, ms:ms + 1])
                    del qk_open[(ms, nt)]

            def qk_chain(ms, nt):
                qk_half(ms, nt, 0)
                qk_half(ms, nt, 1)

            def v_chain(kt):
                v_ps = ps.tile([P, 512], f32, tag="y", bufs=1,
                               name="v_ps")[:, 0:256]
                for ks in range(8):
                    nc.tensor.matmul(
                        v_ps,
                        lhsT=xT_sb[:, ks, kt * 128:(kt + 1) * 128],
                        rhs=wv_sb[:, ks, :],
                        start=(ks == 0), stop=(ks == 7),
                    )
                with nc.allow_low_precision(reason="v bf16 for PE"):
                    nc.vector.tensor_copy(
                        v_sb[:, kt, :, 0:64],
                        v_ps.rearrange("p (h d) -> p h d", h=HPC))

            def op_sub(qb, n):
                # out-projection for one (128-token, 512-outs) tile of block qb
                qt, n2 = n // 2, n % 2
                tok = slice(qb * 512 + qt * 128, qb * 512 + (qt + 1) * 128)
                y_ps = ps.tile([P, 512], f32, tag="y", bufs=1, name="y_ps")
                for ks2 in range(2):
                    nc.tensor.matmul(
                        y_ps,
                        lhsT=oT_sb[:, ks2, tok],
                        rhs=wo_sb[:, ks2, n2 * 512:(n2 + 1) * 512],
                        start=(ks2 == 0), stop=(ks2 == 1),
                    )
                y_sb = sb.tile([P, 512], f32, tag="ysb", bufs=2, name="y_sb")
                nc.vector.tensor_copy(y_sb, y_ps)
                nc.sync.dma_start(out_y[tok, n2 * 512:(n2 + 1) * 512], y_sb)

            # ---- evacuation + deferred normalization ----
            def evac(o_ps):
                # free the accumulator banks with a single PSUM->SBUF copy,
                # then spread the denominators across 128 lanes via SBUF->SBUF
                # DMA so the reciprocal microcode runs in parallel.
                oS = sb.tile([65, 1024], f32, tag="oS", bufs=2, name="oS")
                nc.vector.tensor_copy(oS, o_ps[0:65, :])
                nc.sync.dma_start(dS, oS[64:65, :])
                with nc.allow_low_precision(reason="softmax denom recip"):
                    nc.vector.reciprocal(rS, dS)
                nc.sync.dma_start(r2, rS)
                return oS

            def rbm(p, qb, oS, j):
                # broadcast 1/denom along the 64 channel partitions via a
                # ones-matmul, then normalize head j of pair p into oT.
                rb_ps = ps.tile([P, 512], f32, tag="y", bufs=1,
                                name="rb_ps")[0:64, :]
                nc.tensor.matmul(rb_ps, lhsT=ones_r,
                                 rhs=r2[0:1, j * 512:(j + 1) * 512],
                                 start=True, stop=True)
                rbc_sb = sb.tile([64, 512], f32, tag="rbc", bufs=2, name="rbc_sb")
                nc.vector.tensor_copy(rbc_sb, rb_ps)
                with nc.allow_low_precision(reason="oT f32r for PE"):
                    nc.vector.tensor_tensor(
                        out=oT_sb[64 * j:64 * j + 64, p,
                                  qb * 512:(qb + 1) * 512],
                        in0=oS[0:64, j * 512:(j + 1) * 512],
                        in1=rbc_sb,
                        op=mult,
                    )

            # ---- fixed fill schedule: slot (qb, p, kt) -> PE slack work ----
            fills = {}

            def F(qb, p, kt, fn):
                fills.setdefault((qb, p, kt), []).append(fn)

            for kt in range(KT):
                F(0, 0, kt, lambda kt=kt: v_chain(kt))
            for (ms, nt), s0 in [((2, 1), 3), ((2, 2), 6), ((2, 3), 10),
                                 ((3, 0), 12), ((1, 0), 14)]:
                F(0, 0, s0, lambda ms=ms, nt=nt: qk_half(ms, nt, 0))
                F(0, 0, s0 + 1, lambda ms=ms, nt=nt: qk_half(ms, nt, 1))
            for (ms, nt), s0 in [((3, 1), 1), ((3, 2), 5), ((3, 3), 9),
                                 ((0, 1), 12)]:
                F(0, 1, s0, lambda ms=ms, nt=nt: qk_half(ms, nt, 0))
                F(0, 1, s0 + 1, lambda ms=ms, nt=nt: qk_half(ms, nt, 1))
            for qb in range(1, QB):
                for n in range(4):
                    F(qb, 0, 10 + n, lambda qb=qb, n=n: op_sub(qb - 1, n))
                F(qb, 0, 14, lambda qb=qb: qk_half(1, qb, 0))
                F(qb, 0, 15, lambda qb=qb: qk_half(1, qb, 1))
                for n, s in [(4, 1), (5, 2), (6, 3), (7, 6)]:
                    F(qb, 1, s, lambda qb=qb, n=n: op_sub(qb - 1, n))
                if qb < QB - 1:
                    F(qb, 1, 12, lambda qb=qb: qk_half(0, qb + 1, 0))
                    F(qb, 1, 13, lambda qb=qb: qk_half(0, qb + 1, 1))

            # ---- attention machinery ----
            def scores_exp(qb, p, kt, pend, o_ps):
                key = slice(kt * 128, (kt + 1) * 128)
                q = slice(qb * 512, (qb + 1) * 512)
                s_ps = ps.tile([P, 1024], f32, tag="s", bufs=2, name="s_ps")
                for j in range(2):  # j: head 2p+j on PE rows 64j..64j+63
                    hp = 64 * j
                    nc.tensor.matmul(
                        s_ps[:, j * 512:(j + 1) * 512],
                        lhsT=qkT_sb[hp:hp + 64, 2 + p, key],
                        rhs=qkT_sb[hp:hp + 64, p, q],
                        start=True, stop=True,
                    )
                pT = sb.tile([P, 1024], bf16, tag="pT", bufs=8, name="pT")
                nc.scalar.activation(pT, s_ps, Exp, bias=shift_sb, scale=SCALE)
                pend.append((kt, pT, o_ps, p))

            def pv(pend):
                kt, pT, o_ps, p = pend.pop(0)
                for j in range(2):
                    nc.tensor.matmul(
                        o_ps[0:65, j * 512:(j + 1) * 512],
                        lhsT=v_sb[:, kt, 2 * p + j, :],
                        rhs=pT[:, j * 512:(j + 1) * 512],
                        start=(kt == 0), stop=(kt == KT - 1),
                    )

            # ramp: minimal chains for (0,0) slot 0
            qk_chain(0, 0)
            qk_chain(2, 0)

            pend = []
            prev = None      # (p, qb, oS) awaiting normalization
            for qb in range(QB):
                for p in range(2):
                    o_ps = ps.tile([P, 1024], f32, tag="acc", bufs=1,
                                   name="o_ps")
                    for kt in range(KT):
                        here = fills.get((qb, p, kt), [])
                        for fn in here:
                            fn()
                        if prev is not None and kt in (7, 8):
                            rbm(prev[0], prev[1], prev[2], kt - 7)
                        scores_exp(qb, p, kt, pend, o_ps)
                        if len(pend) > SKEW:
                            pv(pend)
                        if len(pend) > SKEW + 1 and not here:
                            pv(pend)
                    # all of this pair's PV must be emitted before evac reads
                    # (and the next pair's PV overwrites) the accumulator
                    while pend:
                        pv(pend)
                    oS = evac(o_ps)
                    prev = (p, qb, oS)
            rbm(prev[0], prev[1], prev[2], 0)
            rbm(prev[0], prev[1], prev[2], 1)
            for n in range(8):
                op_sub(QB - 1, n)

    nc.compile()
    return nc


def _get_nc():
    if "nc" not in _cache:
        _cache["nc"] = _build_nc()
    return _cache["nc"]


def kernel(x, W_in, b_in, W_out, b_out):
    from concourse.bass_utils import run_bass_kernel_spmd

    x = np.asarray(x, dtype=np.float32)
    W_in = np.asarray(W_in, dtype=np.float32)
    b_in = np.asarray(b_in, dtype=np.float32)
    W_out = np.asarray(W_out, dtype=np.float32)
    b_out = np.asarray(b_out, dtype=np.float32)

    in_maps = []
    for c in range(8):
        b, g = c // 4, c % 4
        rs = slice(256 * g, 256 * g + 256)

        xTc = np.ascontiguousarray(
            x[b].T.reshape(8, 128, N).transpose(1, 0, 2))          # [128,8,N]
        Wqk = np.concatenate([W_in[0:C][rs], W_in[C:2 * C][rs]])   # [512,1024]
        wqkc = np.ascontiguousarray(
            Wqk.T.reshape(8, 128, 512).transpose(1, 0, 2))         # [128,8,512]
        Wv = W_in[2 * C:3 * C][rs]                                 # [256,1024]
        wvc = np.ascontiguousarray(
            Wv.T.reshape(8, 128, 256).transpose(1, 0, 2))          # [128,8,256]
        WoT = np.ascontiguousarray(W_out[:, rs].T)                 # [256,1024]
        woc = np.ascontiguousarray(WoT.reshape(2, 128, 1024).transpose(1, 0, 2))
        bqkc = np.ascontiguousarray(
            np.concatenate([b_in[0:C][rs], b_in[C:2 * C][rs]]).reshape(4, 128).T)

        in_maps.append({"xT": xTc, "wqk": wqkc, "wv": wvc, "wo": woc, "bqk": bqkc})

    nc = _get_nc()
    trace = os.environ.get("KERNEL_TRACE", "0") == "1"
    bkr = run_bass_kernel_spmd(nc, in_maps, core_ids=list(range(8)), trace=trace)
    _cache["last_bkr"] = bkr
    res = bkr.results

    y = np.zeros((B, N, C), dtype=np.float32)
    for c in range(8):
        y[c // 4] += res[c]["out_y"]
    # v-bias folds through softmax (rows sum to 1) and out-proj exactly
    y += (b_in[2 * C:3 * C] @ W_out.T + b_out)[None, None, :]
    return y


# revision 13
# speedup vs baseline: 1.8289x; 1.0779x over previous
"""Trainium2 Bass kernel: 16-head self-attention (B=2, N=2048, C=1024) on 8 cores.

Sharding: core c -> (batch b = c//4, head-group g = c%4 owning heads 4g..4g+3).
Each core computes QKV projection for its heads, full softmax attention, and a
partial out-projection (its heads' input-channel slice of W_out); the host sums
the 4 partials per batch (tensor-parallel all-reduce done on host at gather).

v2 design (vs the first working version, which measured 427us):
  - Head PAIRS processed together with PE row-tiling: the two heads of a pair
    keep their 64 d-channels on partitions 0-63 / 64-127, so their K=64 score
    matmuls run CONCURRENTLY on the two row halves of the PE array.
  - Query-block-major loop (4 blocks of 512 queries): the out-projection of
    block b runs inside block b+1's attention window instead of a serial tail.
  - Softmax denominator reciprocals: [1,512] rows are spread to [128,8] via a
    DRAM round-trip so the DVE reciprocal microcode runs across 128 lanes
    (~0.1us) instead of serializing in one lane (6.5us per call in v1).
  - o accumulators are evacuated PSUM->SBUF immediately (one DVE copy) so the
    single accumulator bank pair can be reused by the next head pair at once;
    normalization (broadcast-matmul + multiply) happens later off SBUF.
  - Emission is software-pipelined: PV runs SKEW slots behind scores/exp, and
    projection / out-projection / normalization quanta are placed in fixed
    slots so the Scalar engine's exp stream (the 128us floor) never starves
    and the PE never idles long enough to re-throttle (HAM).
  - exp on ScalarE reads S^T from PSUM as [128,1024] tiles (head A's 512
    queries | head B's 512): P = exp(scale*S + shift); the constant shift
    cancels in normalization.  All matmuls in float32r (full rate at N>=256);
    probabilities and V in bf16.
"""
import os

import numpy as np

B, N, C, H, D = 2, 2048, 1024, 16, 64
HPC = 4            # heads per core
SCALE = float(D) ** -0.5
SHIFT = -12.0      # exp arg shift; scores observed in [-9.1, 9.1] scaled
P = 128
KT = N // 128      # 16 key tiles
QB = N // 512      # 4 query blocks
SKEW = 4           # PV emission lag (slots) behind scores/exp

_cache = {}


def _build_nc():
    import concourse.bass as bass  # noqa: F401
    import concourse.mybir as mybir
    from concourse import bacc
    from concourse.tile import TileContext

    f32 = mybir.dt.float32
    f32r = mybir.dt.float32r
    bf16 = mybir.dt.bfloat16
    Exp = mybir.ActivationFunctionType.Exp
    mult = mybir.AluOpType.mult

    nc = bacc.Bacc("TRN2", target_bir_lowering=False, debug=False, num_devices=8)

    xT = nc.dram_tensor("xT", [P, 8, N], f32r, kind="ExternalInput")
    wqk = nc.dram_tensor("wqk", [P, 8, 512], f32r, kind="ExternalInput")
    wv = nc.dram_tensor("wv", [P, 8, 256], f32r, kind="ExternalInput")
    wo = nc.dram_tensor("wo", [P, 2, 1024], f32r, kind="ExternalInput")
    bqk = nc.dram_tensor("bqk", [P, 4], f32, kind="ExternalInput")
    out_y = nc.dram_tensor("out_y", [N, C], f32, kind="ExternalOutput")

    with TileContext(nc) as tc:
        with tc.tile_pool(name="sb", bufs=1) as sb, \
             tc.tile_pool(name="ps", bufs=1, space="PSUM") as ps:
            # ---- persistent SBUF ----
            wqk_sb = sb.tile([P, 8, 512], f32r)
            wv_sb = sb.tile([P, 8, 256], f32r)
            wo_sb = sb.tile([P, 2, 1024], f32r)
            bqk_sb = sb.tile([P, 4], f32)
            xT_sb = sb.tile([P, 8, N], f32r)
            qkT_sb = sb.tile([P, 4, N], bf16)       # [q01|q23|k01|k23] x tokens
            v_sb = sb.tile([P, KT, HPC, 65], bf16)  # tokens x (head, D|ones)
            oT_sb = sb.tile([P, 2, N], f32r)        # head channels x tokens
            dS = sb.tile([P, 8], f32)               # denoms spread over lanes
            rS = sb.tile([P, 8], f32r)              # reciprocals, spread
            r2 = sb.tile([1, 1024], f32r)           # reciprocals, heads on free

            # DMA order tuned so just-in-time chains see their inputs arrive:
            # the ramp chains (q pair0, k pair0) need only wqk cols 0:128 /
            # 256:384 plus tokens 0..511, so those weight slices go first.
            for ms in (0, 2):
                nc.sync.dma_start(wqk_sb[:, :, ms * 128:(ms + 1) * 128],
                                  wqk[:, :, ms * 128:(ms + 1) * 128])
            for c in range(2):
                nc.sync.dma_start(xT_sb[:, :, c * 256:(c + 1) * 256],
                                  xT[:, :, c * 256:(c + 1) * 256])
            nc.sync.dma_start(bqk_sb, bqk[:])
            nc.sync.dma_start(wv_sb, wv[:])
            for ms in (1, 3):
                nc.sync.dma_start(wqk_sb[:, :, ms * 128:(ms + 1) * 128],
                                  wqk[:, :, ms * 128:(ms + 1) * 128])
            for c in range(2, 8):
                nc.sync.dma_start(xT_sb[:, :, c * 256:(c + 1) * 256],
                                  xT[:, :, c * 256:(c + 1) * 256])
            nc.sync.dma_start(wo_sb, wo[:])

            ones_f = sb.tile([P, 1], f32)
            nc.vector.memset(ones_f, 1.0)
            with nc.allow_low_precision(reason="exact 1.0 to bf16"):
                nc.vector.tensor_copy(v_sb[:, :, :, 64:65],
                                      ones_f[:, 0:1, None].to_broadcast((P, KT, HPC, 1)))
            ones_r = sb.tile([1, 64], f32r)
            with nc.allow_low_precision(reason="exact 1.0 to f32r"):
                nc.vector.tensor_copy(ones_r, ones_f[0:1, :].to_broadcast((1, 64)))
            shift_sb = sb.tile([P, 1], f32)
            nc.vector.memset(shift_sb, SHIFT)

            # ---- PE slack fillers ----
            qk_open = {}

            def qk_half(ms, nt, half):
                # K=1024 contraction in 8 steps; emitted as two 4-step halves
                # so a chain never monopolizes the PE for >1us.
                tok = slice(nt * 512, (nt + 1) * 512)
                if half == 0:
                    qk_open[(ms, nt)] = ps.tile([P, 512], f32, tag="c", bufs=1,
                                                name="qk_ps")
                qk_ps = qk_open[(ms, nt)]
                for ks in range(4 * half, 4 * half + 4):
                    nc.tensor.matmul(
                        qk_ps,
                        lhsT=wqk_sb[:, ks, ms * 128:(ms + 1) * 128],
                        rhs=xT_sb[:, ks, tok],
                        start=(ks == 0), stop=(ks == 7),
                    )
                if half == 1:
                    with nc.allow_low_precision(reason="qkT f32r for PE"):
                        nc.vector.tensor_scalar_add(
                            qkT_sb[:, ms, tok], qk_ps, bqk_sb[:, ms:ms + 1])
                    del qk_open[(ms, nt)]

            def qk_chain(ms, nt):
                qk_half(ms, nt, 0)
                qk_half(ms, nt, 1)

            def v_chain(kt):
                v_ps = ps.tile([P, 512], f32, tag="y", bufs=1,
                               name="v_ps")[:, 0:256]
                for ks in range(8):
                    nc.tensor.matmul(
                        v_ps,
                        lhsT=xT_sb[:, ks, kt * 128:(kt + 1) * 128],
                        rhs=wv_sb[:, ks, :],
                        start=(ks == 0), stop=(ks == 7),
                    )
                with nc.allow_low_precision(reason="v bf16 for PE"):
                    nc.vector.tensor_copy(
                        v_sb[:, kt, :, 0:64],
                        v_ps.rearrange("p (h d) -> p h d", h=HPC))

            def op_sub(qb, n, tag="y"):
                # out-projection for one (128-token, 512-outs) tile of block qb
                qt, n2 = n // 2, n % 2
                tok = slice(qb * 512 + qt * 128, qb * 512 + (qt + 1) * 128)
                if tag == "y":
                    y_ps = ps.tile([P, 512], f32, tag="y", bufs=1, name="y_ps")
                else:  # tail: the freed double-buffered scores pool pipelines
                    y_ps = ps.tile([P, 1024], f32, tag="s", bufs=2,
                                   name="s_ps")[:, 0:512]
                for ks2 in range(2):
                    nc.tensor.matmul(
                        y_ps,
                        lhsT=oT_sb[:, ks2, tok],
                        rhs=wo_sb[:, ks2, n2 * 512:(n2 + 1) * 512],
                        start=(ks2 == 0), stop=(ks2 == 1),
                    )
                y_sb = sb.tile([P, 512], f32, tag="ysb", bufs=2, name="y_sb")
                nc.vector.tensor_copy(y_sb, y_ps)
                nc.sync.dma_start(out_y[tok, n2 * 512:(n2 + 1) * 512], y_sb)

            # ---- evacuation + deferred normalization ----
            def evac(o_ps):
                # free the accumulator banks with a single PSUM->SBUF copy,
                # then spread the denominators across 128 lanes via SBUF->SBUF
                # DMA so the reciprocal microcode runs in parallel.
                oS = sb.tile([65, 1024], f32, tag="oS", bufs=2, name="oS")
                nc.vector.tensor_copy(oS, o_ps[0:65, :])
                nc.sync.dma_start(dS, oS[64:65, :])
                with nc.allow_low_precision(reason="softmax denom recip"):
                    nc.vector.reciprocal(rS, dS)
                nc.sync.dma_start(r2, rS)
                return oS

            def rbm(p, qb, oS, j):
                # broadcast 1/denom along the 64 channel partitions via a
                # ones-matmul, then normalize head j of pair p into oT.
                rb_ps = ps.tile([P, 512], f32, tag="y", bufs=1,
                                name="rb_ps")[0:64, :]
                nc.tensor.matmul(rb_ps, lhsT=ones_r,
                                 rhs=r2[0:1, j * 512:(j + 1) * 512],
                                 start=True, stop=True)
                rbc_sb = sb.tile([64, 512], f32, tag="rbc", bufs=2, name="rbc_sb")
                nc.vector.tensor_copy(rbc_sb, rb_ps)
                with nc.allow_low_precision(reason="oT f32r for PE"):
                    nc.vector.tensor_tensor(
                        out=oT_sb[64 * j:64 * j + 64, p,
                                  qb * 512:(qb + 1) * 512],
                        in0=oS[0:64, j * 512:(j + 1) * 512],
                        in1=rbc_sb,
                        op=mult,
                    )

            # ---- fixed fill schedule: slot (qb, p, kt) -> PE slack work ----
            fills = {}

            def F(qb, p, kt, fn):
                fills.setdefault((qb, p, kt), []).append(fn)

            for kt in range(KT):
                F(0, 0, kt, lambda kt=kt: v_chain(kt))
            for (ms, nt), s0 in [((2, 1), 3), ((2, 2), 6), ((2, 3), 10),
                                 ((3, 0), 12), ((1, 0), 14)]:
                F(0, 0, s0, lambda ms=ms, nt=nt: qk_half(ms, nt, 0))
                F(0, 0, s0 + 1, lambda ms=ms, nt=nt: qk_half(ms, nt, 1))
            for (ms, nt), s0 in [((3, 1), 1), ((3, 2), 5), ((3, 3), 9),
                                 ((0, 1), 12)]:
                F(0, 1, s0, lambda ms=ms, nt=nt: qk_half(ms, nt, 0))
                F(0, 1, s0 + 1, lambda ms=ms, nt=nt: qk_half(ms, nt, 1))
            for qb in range(1, QB):
                for n in range(4):
                    F(qb, 0, 10 + n, lambda qb=qb, n=n: op_sub(qb - 1, n))
                F(qb, 0, 14, lambda qb=qb: qk_half(1, qb, 0))
                F(qb, 0, 15, lambda qb=qb: qk_half(1, qb, 1))
                if qb < QB - 1:
                    for n, s in [(4, 1), (5, 2), (6, 3), (7, 6)]:
                        F(qb, 1, s, lambda qb=qb, n=n: op_sub(qb - 1, n))
                if qb < QB - 1:
                    F(qb, 1, 12, lambda qb=qb: qk_half(0, qb + 1, 0))
                    F(qb, 1, 13, lambda qb=qb: qk_half(0, qb + 1, 1))

            # ---- attention machinery ----
            def scores_exp(qb, p, kt, pend, o_ps):
                key = slice(kt * 128, (kt + 1) * 128)
                q = slice(qb * 512, (qb + 1) * 512)
                s_ps = ps.tile([P, 1024], f32, tag="s", bufs=2, name="s_ps")
                for j in range(2):  # j: head 2p+j on PE rows 64j..64j+63
                    hp = 64 * j
                    nc.tensor.matmul(
                        s_ps[:, j * 512:(j + 1) * 512],
                        lhsT=qkT_sb[hp:hp + 64, 2 + p, key],
                        rhs=qkT_sb[hp:hp + 64, p, q],
                        start=True, stop=True,
                    )
                pT = sb.tile([P, 1024], bf16, tag="pT", bufs=8, name="pT")
                nc.scalar.activation(pT, s_ps, Exp, bias=shift_sb, scale=SCALE)
                pend.append((kt, pT, o_ps, p))

            def pv(pend):
                kt, pT, o_ps, p = pend.pop(0)
                for j in range(2):
                    nc.tensor.matmul(
                        o_ps[0:65, j * 512:(j + 1) * 512],
                        lhsT=v_sb[:, kt, 2 * p + j, :],
                        rhs=pT[:, j * 512:(j + 1) * 512],
                        start=(kt == 0), stop=(kt == KT - 1),
                    )

            # ramp: minimal chains for (0,0) slot 0
            qk_chain(0, 0)
            qk_chain(2, 0)

            pend = []
            prev = None      # (p, qb, oS) awaiting normalization
            for qb in range(QB):
                for p in range(2):
                    o_ps = ps.tile([P, 1024], f32, tag="acc", bufs=1,
                                   name="o_ps")
                    for kt in range(KT):
                        here = fills.get((qb, p, kt), [])
                        for fn in here:
                            fn()
                        if prev is not None and kt in (7, 8):
                            rbm(prev[0], prev[1], prev[2], kt - 7)
                        scores_exp(qb, p, kt, pend, o_ps)
                        if len(pend) > SKEW:
                            pv(pend)
                        if len(pend) > SKEW + 1 and not here:
                            pv(pend)
                    # all of this pair's PV must be emitted before evac reads
                    # (and the next pair's PV overwrites) the accumulator
                    while pend:
                        pv(pend)
                    oS = evac(o_ps)
                    prev = (p, qb, oS)
            # tail: the deferred qb2 out-proj units fill the PE while the
            # reciprocal DMA round-trip for the last pair is in flight.
            for n in range(4, 8):
                op_sub(QB - 2, n)
            rbm(prev[0], prev[1], prev[2], 0)
            rbm(prev[0], prev[1], prev[2], 1)
            for n in range(8):
                op_sub(QB - 1, n, tag="s")

    nc.compile()
    return nc


def _get_nc():
    if "nc" not in _cache:
        _cache["nc"] = _build_nc()
    return _cache["nc"]


def kernel(x, W_in, b_in, W_out, b_out):
    from concourse.bass_utils import run_bass_kernel_spmd

    x = np.asarray(x, dtype=np.float32)
    W_in = np.asarray(W_in, dtype=np.float32)
    b_in = np.asarray(b_in, dtype=np.float32)
    W_out = np.asarray(W_out, dtype=np.float32)
    b_out = np.asarray(b_out, dtype=np.float32)

    in_maps = []
    for c in range(8):
        b, g = c // 4, c % 4
        rs = slice(256 * g, 256 * g + 256)

        xTc = np.ascontiguousarray(
            x[b].T.reshape(8, 128, N).transpose(1, 0, 2))          # [128,8,N]
        Wqk = np.concatenate([W_in[0:C][rs], W_in[C:2 * C][rs]])   # [512,1024]
        wqkc = np.ascontiguousarray(
            Wqk.T.reshape(8, 128, 512).transpose(1, 0, 2))         # [128,8,512]
        Wv = W_in[2 * C:3 * C][rs]                                 # [256,1024]
        wvc = np.ascontiguousarray(
            Wv.T.reshape(8, 128, 256).transpose(1, 0, 2))          # [128,8,256]
        WoT = np.ascontiguousarray(W_out[:, rs].T)                 # [256,1024]
        woc = np.ascontiguousarray(WoT.reshape(2, 128, 1024).transpose(1, 0, 2))
        bqkc = np.ascontiguousarray(
            np.concatenate([b_in[0:C][rs], b_in[C:2 * C][rs]]).reshape(4, 128).T)

        in_maps.append({"xT": xTc, "wqk": wqkc, "wv": wvc, "wo": woc, "bqk": bqkc})

    nc = _get_nc()
    trace = os.environ.get("KERNEL_TRACE", "0") == "1"
    bkr = run_bass_kernel_spmd(nc, in_maps, core_ids=list(range(8)), trace=trace)
    _cache["last_bkr"] = bkr
    res = bkr.results

    y = np.zeros((B, N, C), dtype=np.float32)
    for c in range(8):
        y[c // 4] += res[c]["out_y"]
    # v-bias folds through softmax (rows sum to 1) and out-proj exactly
    y += (b_in[2 * C:3 * C] @ W_out.T + b_out)[None, None, :]
    return y


# revision 17
# speedup vs baseline: 1.9324x; 1.0566x over previous
"""Trainium2 Bass kernel: 16-head self-attention (B=2, N=2048, C=1024) on 8 cores.

Sharding: core c -> (batch b = c//4, head-group g = c%4 owning heads 4g..4g+3).
Each core computes QKV projection for its heads, full softmax attention, and a
partial out-projection (its heads' input-channel slice of W_out); the host sums
the 4 partials per batch (tensor-parallel all-reduce done on host at gather).

v2 design (vs the first working version, which measured 427us):
  - Head PAIRS processed together with PE row-tiling: the two heads of a pair
    keep their 64 d-channels on partitions 0-63 / 64-127, so their K=64 score
    matmuls run CONCURRENTLY on the two row halves of the PE array.
  - Query-block-major loop (4 blocks of 512 queries): the out-projection of
    block b runs inside block b+1's attention window instead of a serial tail.
  - Softmax denominator reciprocals: [1,512] rows are spread to [128,8] via a
    DRAM round-trip so the DVE reciprocal microcode runs across 128 lanes
    (~0.1us) instead of serializing in one lane (6.5us per call in v1).
  - o accumulators are evacuated PSUM->SBUF immediately (one DVE copy) so the
    single accumulator bank pair can be reused by the next head pair at once;
    normalization (broadcast-matmul + multiply) happens later off SBUF.
  - Emission is software-pipelined: PV runs SKEW slots behind scores/exp, and
    projection / out-projection / normalization quanta are placed in fixed
    slots so the Scalar engine's exp stream (the 128us floor) never starves
    and the PE never idles long enough to re-throttle (HAM).
  - exp on ScalarE reads S^T from PSUM as [128,1024] tiles (head A's 512
    queries | head B's 512): P = exp(scale*S + shift); the constant shift
    cancels in normalization.  All matmuls in float32r (full rate at N>=256);
    probabilities and V in bf16.
"""
import os

import numpy as np

B, N, C, H, D = 2, 2048, 1024, 16, 64
HPC = 4            # heads per core
SCALE = float(D) ** -0.5
SHIFT = -12.0      # exp arg shift; scores observed in [-9.1, 9.1] scaled
P = 128
KT = N // 128      # 16 key tiles
QB = N // 512      # 4 query blocks
SKEW = 4           # PV emission lag (slots) behind scores/exp

_cache = {}


def _build_nc():
    import concourse.bass as bass  # noqa: F401
    import concourse.mybir as mybir
    from concourse import bacc
    from concourse.tile import TileContext

    f32 = mybir.dt.float32
    f32r = mybir.dt.float32r
    bf16 = mybir.dt.bfloat16
    Exp = mybir.ActivationFunctionType.Exp
    mult = mybir.AluOpType.mult

    nc = bacc.Bacc("TRN2", target_bir_lowering=False, debug=False, num_devices=8)

    xT = nc.dram_tensor("xT", [P, 8, N], bf16, kind="ExternalInput")
    wqk = nc.dram_tensor("wqk", [P, 8, 512], bf16, kind="ExternalInput")
    wv = nc.dram_tensor("wv", [P, 8, 256], bf16, kind="ExternalInput")
    wo = nc.dram_tensor("wo", [P, 2, 1024], f32r, kind="ExternalInput")
    bqk = nc.dram_tensor("bqk", [P, 4], f32, kind="ExternalInput")
    out_y = nc.dram_tensor("out_y", [N, C], f32, kind="ExternalOutput")

    with TileContext(nc) as tc:
        with tc.tile_pool(name="sb", bufs=1) as sb, \
             tc.tile_pool(name="ps", bufs=1, space="PSUM") as ps:
            # ---- persistent SBUF ----
            wqk_sb = sb.tile([P, 8, 512], bf16)
            wv_sb = sb.tile([P, 8, 256], bf16)
            wo_sb = sb.tile([P, 2, 1024], f32r)
            bqk_sb = sb.tile([P, 4], f32)
            xT_sb = sb.tile([P, 8, N], bf16)
            qkT_sb = sb.tile([P, 4, N], bf16)       # [q01|q23|k01|k23] x tokens
            v_sb = sb.tile([P, KT, HPC, 65], bf16)  # tokens x (head, D|ones)
            oT_sb = sb.tile([P, 2, N], f32r)        # head channels x tokens
            dS = sb.tile([P, 8], f32)               # denoms spread over lanes
            rS = sb.tile([P, 8], f32r)              # reciprocals, spread
            r2 = sb.tile([1, 1024], f32r)           # reciprocals, heads on free

            # DMA order tuned so just-in-time chains see their inputs arrive:
            # the ramp chains (q pair0, k pair0) need only wqk cols 0:128 /
            # 256:384 plus tokens 0..511, so those weight slices go first.
            for ms in (0, 2):
                nc.sync.dma_start(wqk_sb[:, :, ms * 128:(ms + 1) * 128],
                                  wqk[:, :, ms * 128:(ms + 1) * 128])
            for c in range(2):
                nc.sync.dma_start(xT_sb[:, :, c * 256:(c + 1) * 256],
                                  xT[:, :, c * 256:(c + 1) * 256])
            nc.sync.dma_start(bqk_sb, bqk[:])
            nc.sync.dma_start(wv_sb, wv[:])
            for ms in (1, 3):
                nc.sync.dma_start(wqk_sb[:, :, ms * 128:(ms + 1) * 128],
                                  wqk[:, :, ms * 128:(ms + 1) * 128])
            for c in range(2, 8):
                nc.sync.dma_start(xT_sb[:, :, c * 256:(c + 1) * 256],
                                  xT[:, :, c * 256:(c + 1) * 256])
            nc.sync.dma_start(wo_sb, wo[:])

            ones_f = sb.tile([P, 1], f32)
            nc.vector.memset(ones_f, 1.0)
            with nc.allow_low_precision(reason="exact 1.0 to bf16"):
                nc.vector.tensor_copy(v_sb[:, :, :, 64:65],
                                      ones_f[:, 0:1, None].to_broadcast((P, KT, HPC, 1)))
            ones_r = sb.tile([1, 64], f32r)
            with nc.allow_low_precision(reason="exact 1.0 to f32r"):
                nc.vector.tensor_copy(ones_r, ones_f[0:1, :].to_broadcast((1, 64)))
            shift_sb = sb.tile([P, 1], f32)
            nc.vector.memset(shift_sb, SHIFT)

            # ---- PE slack fillers ----
            qk_open = {}

            def qk_half(ms, nt, half):
                # K=1024 contraction in 8 steps; emitted as two 4-step halves
                # so a chain never monopolizes the PE for >1us.
                tok = slice(nt * 512, (nt + 1) * 512)
                if half == 0:
                    qk_open[(ms, nt)] = ps.tile([P, 512], f32, tag="c", bufs=1,
                                                name="qk_ps")
                qk_ps = qk_open[(ms, nt)]
                for ks in range(4 * half, 4 * half + 4):
                    nc.tensor.matmul(
                        qk_ps,
                        lhsT=wqk_sb[:, ks, ms * 128:(ms + 1) * 128],
                        rhs=xT_sb[:, ks, tok],
                        start=(ks == 0), stop=(ks == 7),
                    )
                if half == 1:
                    with nc.allow_low_precision(reason="qkT f32r for PE"):
                        nc.vector.tensor_scalar_add(
                            qkT_sb[:, ms, tok], qk_ps, bqk_sb[:, ms:ms + 1])
                    del qk_open[(ms, nt)]

            def qk_chain(ms, nt):
                qk_half(ms, nt, 0)
                qk_half(ms, nt, 1)

            def v_chain(kt):
                v_ps = ps.tile([P, 512], f32, tag="y", bufs=1,
                               name="v_ps")[:, 0:256]
                for ks in range(8):
                    nc.tensor.matmul(
                        v_ps,
                        lhsT=xT_sb[:, ks, kt * 128:(kt + 1) * 128],
                        rhs=wv_sb[:, ks, :],
                        start=(ks == 0), stop=(ks == 7),
                    )
                with nc.allow_low_precision(reason="v bf16 for PE"):
                    nc.vector.tensor_copy(
                        v_sb[:, kt, :, 0:64],
                        v_ps.rearrange("p (h d) -> p h d", h=HPC))

            def op_sub(qb, n, tag="y"):
                # out-projection for one (128-token, 512-outs) tile of block qb
                qt, n2 = n // 2, n % 2
                tok = slice(qb * 512 + qt * 128, qb * 512 + (qt + 1) * 128)
                if tag == "y":
                    y_ps = ps.tile([P, 512], f32, tag="y", bufs=1, name="y_ps")
                else:  # tail: the freed double-buffered scores pool pipelines
                    y_ps = ps.tile([P, 1024], f32, tag="s", bufs=2,
                                   name="s_ps")[:, 0:512]
                for ks2 in range(2):
                    nc.tensor.matmul(
                        y_ps,
                        lhsT=oT_sb[:, ks2, tok],
                        rhs=wo_sb[:, ks2, n2 * 512:(n2 + 1) * 512],
                        start=(ks2 == 0), stop=(ks2 == 1),
                    )
                y_sb = sb.tile([P, 512], f32, tag="ysb", bufs=2, name="y_sb")
                nc.vector.tensor_copy(y_sb, y_ps)
                nc.sync.dma_start(out_y[tok, n2 * 512:(n2 + 1) * 512], y_sb)

            # ---- evacuation + deferred normalization ----
            def evac(o_ps):
                # free the accumulator banks with a single PSUM->SBUF copy,
                # then spread the denominators across 128 lanes via SBUF->SBUF
                # DMA so the reciprocal microcode runs in parallel.
                oS = sb.tile([65, 1024], f32, tag="oS", bufs=2, name="oS")
                nc.vector.tensor_copy(oS, o_ps[0:65, :])
                nc.gpsimd.dma_start(dS, oS[64:65, :])
                with nc.allow_low_precision(reason="softmax denom recip"):
                    nc.vector.reciprocal(rS, dS)
                nc.gpsimd.dma_start(r2, rS)
                return oS

            def rbm(p, qb, oS, j):
                # broadcast 1/denom along the 64 channel partitions via a
                # ones-matmul, then normalize head j of pair p into oT.
                rb_ps = ps.tile([P, 512], f32, tag="y", bufs=1,
                                name="rb_ps")[0:64, :]
                nc.tensor.matmul(rb_ps, lhsT=ones_r,
                                 rhs=r2[0:1, j * 512:(j + 1) * 512],
                                 start=True, stop=True)
                rbc_sb = sb.tile([64, 512], f32, tag="rbc", bufs=2, name="rbc_sb")
                nc.vector.tensor_copy(rbc_sb, rb_ps)
                with nc.allow_low_precision(reason="oT f32r for PE"):
                    nc.vector.tensor_tensor(
                        out=oT_sb[64 * j:64 * j + 64, p,
                                  qb * 512:(qb + 1) * 512],
                        in0=oS[0:64, j * 512:(j + 1) * 512],
                        in1=rbc_sb,
                        op=mult,
                    )

            # ---- fixed fill schedule: slot (qb, p, kt) -> PE slack work ----
            fills = {}

            def F(qb, p, kt, fn):
                fills.setdefault((qb, p, kt), []).append(fn)

            for kt in range(KT):
                F(0, 0, kt, lambda kt=kt: v_chain(kt))
            for (ms, nt), s0 in [((2, 1), 3), ((2, 2), 6), ((2, 3), 10),
                                 ((3, 0), 12), ((1, 0), 14)]:
                F(0, 0, s0, lambda ms=ms, nt=nt: qk_half(ms, nt, 0))
                F(0, 0, s0 + 1, lambda ms=ms, nt=nt: qk_half(ms, nt, 1))
            for (ms, nt), s0 in [((3, 1), 1), ((3, 2), 5), ((3, 3), 9),
                                 ((0, 1), 12)]:
                F(0, 1, s0, lambda ms=ms, nt=nt: qk_half(ms, nt, 0))
                F(0, 1, s0 + 1, lambda ms=ms, nt=nt: qk_half(ms, nt, 1))
            for qb in range(1, QB):
                for n in range(4):
                    F(qb, 0, 10 + n, lambda qb=qb, n=n: op_sub(qb - 1, n))
                F(qb, 0, 14, lambda qb=qb: qk_half(1, qb, 0))
                F(qb, 0, 15, lambda qb=qb: qk_half(1, qb, 1))
                if qb < QB - 1:
                    for n, s in [(4, 1), (5, 2), (6, 3), (7, 6)]:
                        F(qb, 1, s, lambda qb=qb, n=n: op_sub(qb - 1, n))
                if qb < QB - 1:
                    F(qb, 1, 12, lambda qb=qb: qk_half(0, qb + 1, 0))
                    F(qb, 1, 13, lambda qb=qb: qk_half(0, qb + 1, 1))

            # ---- attention machinery ----
            def scores_exp(qb, p, kt, pend, o_ps):
                key = slice(kt * 128, (kt + 1) * 128)
                q = slice(qb * 512, (qb + 1) * 512)
                s_ps = ps.tile([P, 1024], f32, tag="s", bufs=2, name="s_ps")
                for j in range(2):  # j: head 2p+j on PE rows 64j..64j+63
                    hp = 64 * j
                    nc.tensor.matmul(
                        s_ps[:, j * 512:(j + 1) * 512],
                        lhsT=qkT_sb[hp:hp + 64, 2 + p, key],
                        rhs=qkT_sb[hp:hp + 64, p, q],
                        start=True, stop=True,
                    )
                pT = sb.tile([P, 1024], bf16, tag="pT", bufs=8, name="pT")
                nc.scalar.activation(pT, s_ps, Exp, bias=shift_sb, scale=SCALE)
                pend.append((kt, pT, o_ps, p))

            def pv(pend):
                kt, pT, o_ps, p = pend.pop(0)
                for j in range(2):
                    nc.tensor.matmul(
                        o_ps[0:65, j * 512:(j + 1) * 512],
                        lhsT=v_sb[:, kt, 2 * p + j, :],
                        rhs=pT[:, j * 512:(j + 1) * 512],
                        start=(kt == 0), stop=(kt == KT - 1),
                    )

            # ramp: minimal chains for (0,0) slot 0
            qk_chain(0, 0)
            qk_chain(2, 0)

            pend = []
            prev = None       # (p, qb, oS) awaiting normalization
            prev_acc = None   # previous pair's accumulator + identity
            for qb in range(QB):
                for p in range(2):
                    o_ps = ps.tile([P, 1024], f32, tag="acc", bufs=1,
                                   name="o_ps")
                    for kt in range(KT):
                        here = fills.get((qb, p, kt), [])
                        for fn in here:
                            fn()
                        if prev is not None and kt in (7, 8):
                            rbm(prev[0], prev[1], prev[2], kt - 7)
                        scores_exp(qb, p, kt, pend, o_ps)
                        if kt == 0 and prev_acc is not None:
                            # the previous pair's remaining PV + evacuation is
                            # emitted after this pair's first scores/exp so the
                            # Scalar engine keeps streaming over the boundary
                            pp, pqb, po = prev_acc
                            while pend and pend[0][2] is po:
                                pv(pend)
                            prev = (pp, pqb, evac(po))
                        while len(pend) > SKEW:
                            pv(pend)
                    prev_acc = (p, qb, o_ps)
            # tail: drain the last pair; deferred qb2 out-proj units fill the
            # PE while the reciprocal DMA round-trip is in flight.
            while pend:
                pv(pend)
            oS = evac(o_ps)
            for n in range(4, 8):
                op_sub(QB - 2, n)
            rbm(prev_acc[0], prev_acc[1], oS, 0)
            rbm(prev_acc[0], prev_acc[1], oS, 1)
            for n in range(8):
                op_sub(QB - 1, n, tag="s")

    nc.compile()
    return nc


def _get_nc():
    if "nc" not in _cache:
        _cache["nc"] = _build_nc()
    return _cache["nc"]


def kernel(x, W_in, b_in, W_out, b_out):
    import ml_dtypes
    from concourse.bass_utils import run_bass_kernel_spmd

    bf16 = ml_dtypes.bfloat16
    x = np.asarray(x, dtype=np.float32)
    W_in = np.asarray(W_in, dtype=np.float32)
    b_in = np.asarray(b_in, dtype=np.float32)
    W_out = np.asarray(W_out, dtype=np.float32)
    b_out = np.asarray(b_out, dtype=np.float32)

    in_maps = []
    for c in range(8):
        b, g = c // 4, c % 4
        rs = slice(256 * g, 256 * g + 256)

        xTc = np.ascontiguousarray(
            x[b].T.reshape(8, 128, N).transpose(1, 0, 2)).astype(bf16)
        Wqk = np.concatenate([W_in[0:C][rs], W_in[C:2 * C][rs]])   # [512,1024]
        wqkc = np.ascontiguousarray(
            Wqk.T.reshape(8, 128, 512).transpose(1, 0, 2)).astype(bf16)
        Wv = W_in[2 * C:3 * C][rs]                                 # [256,1024]
        wvc = np.ascontiguousarray(
            Wv.T.reshape(8, 128, 256).transpose(1, 0, 2)).astype(bf16)
        WoT = np.ascontiguousarray(W_out[:, rs].T)                 # [256,1024]
        woc = np.ascontiguousarray(WoT.reshape(2, 128, 1024).transpose(1, 0, 2))
        bqkc = np.ascontiguousarray(
            np.concatenate([b_in[0:C][rs], b_in[C:2 * C][rs]]).reshape(4, 128).T)

        in_maps.append({"xT": xTc, "wqk": wqkc, "wv": wvc, "wo": woc, "bqk": bqkc})

    nc = _get_nc()
    trace = os.environ.get("KERNEL_TRACE", "0") == "1"
    bkr = run_bass_kernel_spmd(nc, in_maps, core_ids=list(range(8)), trace=trace)
    _cache["last_bkr"] = bkr
    res = bkr.results

    y = np.zeros((B, N, C), dtype=np.float32)
    for c in range(8):
        y[c // 4] += res[c]["out_y"]
    # v-bias folds through softmax (rows sum to 1) and out-proj exactly
    y += (b_in[2 * C:3 * C] @ W_out.T + b_out)[None, None, :]
    return y
